# revision 41
# baseline (speedup 1.0000x reference)
"""Multi-head attention (B=4, L=2048, D=512, H=8) on 8 Trainium2 cores.

Sharding: core c handles batch b = c//2, query rows [(c%2)*1024, +1024).
The key-padding mask knocks out ~half of all kv positions, so the host
compresses K/V to the unmasked positions only (padded to a multiple of
128, kvpc chunks); each core projects the full compressed K/V for its
batch (no collectives).  All matmuls are bf16 (fp8 attn was tried: its
~2.4% rms quantization error lands right at the 2e-2 tolerance).

Key scheduling facts (measured on hw):
 - matmul time = N(out free) cycles only; K=64 matmuls carry a ~100ns
   penalty, so scores use per-head zero-padded q (qTz) for full K=128
 - the exp ACT ops (~1.07us per [128,1024] chunk) and the tensor queue
   are co-critical: projections are emitted as deadline-placed filler
   between score/attnV units so neither engine starves
 - each dma_start costs ~620ns of sync-queue issue; inputs are a few
   host-pre-laid [128, chunk, cols] tensors
 - output projection is split hp0+hp1 (filler, window 5-6) / hp2
   (window 7) / hp3 (tail) via an SBUF accumulator

Device layouts (per core):
  xq/xk/xv_all (128, 4, LQ|KVP)  inputs, dmodel chunk on dim 1
  qTz (128, LQ) x8               head h's q at partitions 64*(h%2), rest 0
  kT (128, KVP) x4               head pair hp at partition halves
  V (128, 8*128) per kv chunk    per head: 64 V cols then 64 ones cols
                                 (denominator rows come out replicated)
  at (128, 1024) bf16            exp(attn) for one kv chunk
  ss (128kv, 1024q) PSUM         scores; exp+mask+scale in one ACT op
  xs (128, 512) PSUM             attn@V accum; rows 64-127 = softmax denom
"""
import math

import numpy as np
import ml_dtypes

import concourse.bacc as bacc
import concourse.bass as bass
import concourse.mybir as mybir
import concourse.tile as tile
from concourse.bass_utils import run_bass_kernel_spmd

F32 = mybir.dt.float32
BF16 = mybir.dt.bfloat16
AF = mybir.ActivationFunctionType

B, L, D = 4, 2048, 512
H, DK = 8, 64
N_CORES = 8
LQ = L // 2            # query rows per core
P = 128
MC = D // P            # 4 dmodel chunks
MASK_BIAS = np.float32(-1e30)
EXP_SHIFT = np.float32(0.0)

_cache = {}


def _build(kvpc):
    """kvpc = number of 128-wide kv chunks after mask compression."""
    kvp = kvpc * P

    nc = bacc.Bacc("TRN2", target_bir_lowering=False, debug=False,
                   num_devices=N_CORES)

    # inputs are host-laid-out partition-major ([128, chunk, cols]) so each
    # loads with ONE dma_start (sync-queue issue slots are ~620ns each);
    # wq/wk are additionally m-major so the m=0 slice can load first
    xqT_d = nc.dram_tensor("xqT", [P, MC, LQ], BF16, kind="ExternalInput").ap()
    xkT_d = nc.dram_tensor("xkT", [P, MC, kvp], BF16, kind="ExternalInput").ap()
    xvT_d = nc.dram_tensor("xvT", [P, MC, kvp], BF16, kind="ExternalInput").ap()
    wq_d = nc.dram_tensor("wq", [P, MC, D], BF16, kind="ExternalInput").ap()
    wk_d = nc.dram_tensor("wk", [P, MC, D], BF16, kind="ExternalInput").ap()
    wv_d = nc.dram_tensor("wv", [P, MC, D], BF16, kind="ExternalInput").ap()
    wo_d = nc.dram_tensor("wo", [P, MC, D], BF16, kind="ExternalInput").ap()
    bq_d = nc.dram_tensor("bq", [P, MC], F32, kind="ExternalInput").ap()
    bk_d = nc.dram_tensor("bk", [P, MC], F32, kind="ExternalInput").ap()
    bv_d = nc.dram_tensor("bv", [1, D], F32, kind="ExternalInput").ap()
    bo_d = nc.dram_tensor("bo", [1, D], F32, kind="ExternalInput").ap()
    mb_d = nc.dram_tensor("mb", [P, kvpc], F32, kind="ExternalInput").ap()
    out_d = nc.dram_tensor("out", [LQ, D], F32, kind="ExternalOutput").ap()

    kblocks = []
    off = 0
    while off < kvp:
        sz = min(512, kvp - off)
        kblocks.append((off, sz))
        off += sz

    with tile.TileContext(nc) as tc:
        with tc.tile_pool(name="const", bufs=1) as cpool, \
             tc.tile_pool(name="xin", bufs=1) as xpool, \
             tc.tile_pool(name="proj", bufs=1) as prpool, \
             tc.tile_pool(name="attn", bufs=3) as apool, \
             tc.tile_pool(name="norm", bufs=4) as npool, \
             tc.tile_pool(name="outp", bufs=3) as opool, \
             tc.tile_pool(name="ps", bufs=2, space="PSUM") as ps:

            # tiles declared up front; DMAs emitted in first-use order below
            wq_all = cpool.tile([P, MC, D], BF16, name="wq_all")
            wk_all = cpool.tile([P, MC, D], BF16, name="wk_all")
            wv_all = cpool.tile([P, MC, D], BF16, name="wv_all")
            wo_all = cpool.tile([P, MC, D], BF16, name="wo_all")
            xq_all = xpool.tile([P, MC, LQ], BF16, name="xq_all")
            xk_all = xpool.tile([P, MC, kvp], BF16, name="xk_all")
            xv_all = xpool.tile([P, MC, kvp], BF16, name="xv_all")
            xqT = [xq_all[:, kc, :] for kc in range(MC)]
            xkT = [xk_all[:, kc, :] for kc in range(MC)]
            xvT = [xv_all[:, kc, :] for kc in range(MC)]
            wv = [wv_all[:, kc, :] for kc in range(MC)]
            wo = [wo_all[:, kc, :] for kc in range(MC)]
            wqm = [wq_all[:, m, :] for m in range(MC)]
            wkm = [wk_all[:, m, :] for m in range(MC)]

            nc.sync.dma_start(wq_all[:, 0, :], wq_d[:, 0, :])
            nc.sync.dma_start(xq_all[:, :, 0:512], xqT_d[:, :, 0:512])
            nc.sync.dma_start(xq_all[:, :, 512:1024], xqT_d[:, :, 512:1024])
            bq = cpool.tile_from(bq_d)
            bk = cpool.tile_from(bk_d)
            mb = cpool.tile_from(mb_d)
            nc.sync.dma_start(wk_all[:, 0, :], wk_d[:, 0, :])
            nc.sync.dma_start(xk_all[:, :, 0:512], xkT_d[:, :, 0:512])
            nc.sync.dma_start(wv_all[:], wv_d)
            nc.sync.dma_start(xv_all[:], xvT_d)
            if kvp > 512:
                nc.sync.dma_start(xk_all[:, :, 512:kvp], xkT_d[:, :, 512:kvp])
            nc.sync.dma_start(wq_all[:, 1:MC, :], wq_d[:, 1:MC, :])
            nc.sync.dma_start(wk_all[:, 1:MC, :], wk_d[:, 1:MC, :])
            bv = cpool.tile_from(bv_d)
            bo = cpool.tile_from(bo_d)
            bv_bc = cpool.tile([P, D], F32)
            nc.gpsimd.partition_broadcast(bv_bc[:], bv[:])
            bo_bc = cpool.tile([P, D], F32)
            nc.gpsimd.partition_broadcast(bo_bc[:], bo[:])
            nc.sync.dma_start(wo_all[:], wo_d)

            qTz = [prpool.tile([P, LQ], BF16, tag=f"qz{h}", name=f"qz{h}")
                   for h in range(H)]
            zeroed = set()
            kT = [prpool.tile([P, kvp], BF16, tag=f"kT{m}", name=f"kT{m}")
                  for m in range(MC)]
            V = [prpool.tile([P, H * P], BF16, tag=f"V{t}", name=f"V{t}")
                 for t in range(kvpc)]
            xsT2 = [prpool.tile([P, LQ], BF16, tag=f"xs{hp}",
                                name=f"xsT2_{hp}") for hp in range(MC)]

            def qproj_blk(m, off, sz):
                if m not in zeroed:  # zero the other head's half once
                    zeroed.add(m)
                    nc.vector.memset(qTz[2 * m][64:128, :], 0.0)
                    nc.vector.memset(qTz[2 * m + 1][0:64, :], 0.0)
                wv4 = wqm[m].rearrange("p (kc mc) -> p kc mc", mc=P)
                pp = ps.tile([P, 512], F32, tag="proj")
                for kc in range(MC):
                    nc.tensor.matmul(
                        pp[:, 0:sz], wv4[:, kc, :],
                        xqT[kc][:, off:off + sz],
                        start=kc == 0, stop=kc == MC - 1)
                nc.vector.tensor_scalar_add(qTz[2 * m][0:64, off:off + sz],
                                            pp[0:64, 0:sz], bq[0:64, m:m + 1])
                nc.vector.tensor_scalar_add(
                    qTz[2 * m + 1][64:128, off:off + sz],
                    pp[64:128, 0:sz], bq[64:128, m:m + 1])

            def kproj_blk(m, off, sz):
                wv4 = wkm[m].rearrange("p (kc mc) -> p kc mc", mc=P)
                pp = ps.tile([P, 512], F32, tag="proj")
                for kc in range(MC):
                    nc.tensor.matmul(
                        pp[:, 0:sz], wv4[:, kc, :],
                        xkT[kc][:, off:off + sz],
                        start=kc == 0, stop=kc == MC - 1)
                nc.vector.tensor_scalar_add(kT[m][:, off:off + sz],
                                            pp[:, 0:sz], bk[:, m:m + 1])

            def qproj(m):
                for off, sz in ((0, 512), (512, 512)):
                    qproj_blk(m, off, sz)

            def kproj(m):
                for off, sz in kblocks:
                    kproj_blk(m, off, sz)

            def vproj(t):
                pv = ps.tile([P, 512], F32, tag="proj")
                for kc in range(MC):
                    nc.tensor.matmul(pv[:], xvT[kc][:, t * P:(t + 1) * P],
                                     wv[kc][:, :], start=kc == 0,
                                     stop=kc == MC - 1)
                v8 = V[t].rearrange("p (g d) -> p g d", d=P)
                nc.vector.tensor_add(v8[:, :, 0:64],
                                     pv.rearrange("p (g d) -> p g d", d=64),
                                     bv_bc.rearrange("p (g d) -> p g d", d=64))
                nc.vector.memset(v8[:, :, 64:P], 1.0)

            def score_chunk(h, c, at):
                hp, po = h // 2, 64 * (h % 2)
                ss = ps.tile([P, 1024], F32, tag="scores")
                for qh in range(2):
                    nc.tensor.matmul(
                        ss[:, qh * 512:(qh + 1) * 512],
                        kT[hp][:, c * P:(c + 1) * P],
                        qTz[h][:, qh * 512:(qh + 1) * 512],
                        start=True, stop=True)
                nc.scalar.activation(at[c][:], ss[:], AF.Exp,
                                     bias=mb[:, c:c + 1], scale=0.125)

            def attnv_units(h, at):
                """Yield emission thunks for attn@V + normalize of head h."""
                hp, po = h // 2, 64 * (h % 2)
                xs = [None, None]

                def mk_mm(qh, c):
                    def emit():
                        if c == 0:
                            xs[qh] = ps.tile([P, 512], F32, tag="xs",
                                             name=f"xs_h{h}_{qh}")
                        nc.tensor.matmul(
                            xs[qh][:], V[c][:, P * h:P * h + P],
                            at[c][:, qh * 512:(qh + 1) * 512],
                            start=c == 0, stop=c == kvpc - 1)
                    return emit

                def mk_norm(qh):
                    def emit():
                        srow = npool.tile([64, 512], F32, tag="srow")
                        nc.vector.tensor_copy(srow[:], xs[qh][64:128, :])
                        rec = npool.tile([64, 512], F32, tag="rec")
                        nc.vector.reciprocal_approx_fast(rec[:], srow[:])
                        nc.vector.tensor_mul(
                            xsT2[hp][po:po + 64, qh * 512:(qh + 1) * 512],
                            xs[qh][0:64, :], rec[:])
                    return emit

                for qh in range(2):
                    for c in range(kvpc):
                        yield mk_mm(qh, c)
                    yield mk_norm(qh)

            def new_at(h):
                return [apool.tile([P, 1024], BF16, tag=f"at{c}",
                                   name=f"at_h{h}_{c}")
                        for c in range(kvpc)]

            # ---- emission schedule ----
            # head 0: kproj blocks land just before the score chunks that
            # need them; vproj rides along so V is ready for attnV(0)
            qproj(0)
            ats = {0: new_at(0)}
            nb = 0
            for c in range(kvpc):
                if c * P >= nb:  # next kproj block
                    bi = len([1 for o, s in kblocks if o < nb])
                    off, sz = kblocks[bi]
                    kproj_blk(0, off, sz)
                    nb = off + sz
                score_chunk(0, c, ats[0])

            # output projection is split: hp0/hp1 partials run as mid-kernel
            # filler into an SBUF accumulator (bias folded in); only hp2/hp3
            # + one DVE add remain for the tail
            oacc = [prpool.tile([P, D], F32, tag=f"oacc{qt}",
                                name=f"oacc{qt}") for qt in range(LQ // P)]

            def opa(qt):
                pA = ps.tile([P, 512], F32, tag="proj")
                for hp in (0, 1):
                    nc.tensor.matmul(pA[:], xsT2[hp][:, qt * P:(qt + 1) * P],
                                     wo[hp][:, :], start=hp == 0, stop=hp == 1)
                nc.vector.tensor_add(oacc[qt][:], pA[:], bo_bc[:])

            def opb(qt):
                pB = ps.tile([P, 512], F32, tag="proj")
                nc.tensor.matmul(pB[:], xsT2[2][:, qt * P:(qt + 1) * P],
                                 wo[2][:, :], start=True, stop=True)
                nc.vector.tensor_add(oacc[qt][:], pB[:], oacc[qt][:])

            def mk_qk(m):
                return ([(qproj_blk, m, 0, 512), (qproj_blk, m, 512, 512)] +
                        [(kproj_blk, m, o, s) for o, s in kblocks])

            # filler placed by deadline: qk(m) lands a window before the
            # heads that read it; oproj partials go late (after head 3)
            ats[1] = new_at(1)
            qk1 = mk_qk(1)
            for c in range(kvpc):
                score_chunk(1, c, ats[1])
                vproj(c)
                if c < len(qk1):
                    fn, m, off, sz = qk1[c]
                    fn(m, off, sz)
            for f in qk1[kvpc:]:
                fn, m, off, sz = f
                fn(m, off, sz)

            filler = {2: mk_qk(2), 4: mk_qk(3),
                      5: [(opa, qt) for qt in range(4)],
                      6: [(opa, qt) for qt in range(4, 8)],
                      7: [(opb, qt) for qt in range(8)]}

            for h in range(1, H):
                if h < H - 1:
                    ats[h + 1] = new_at(h + 1)
                units = list(attnv_units(h - 1, ats[h - 1]))
                fl = filler.get(h, [])
                ui = 0
                for c in range(kvpc):
                    if c < len(fl):
                        if fl[c][0] in (opa, opb):
                            fl[c][0](fl[c][1])
                        else:
                            fn, m, off, sz = fl[c]
                            fn(m, off, sz)
                    if h < H - 1:
                        score_chunk(h + 1, c, ats[h + 1])
                    take = 2 if c < kvpc - 1 else len(units) - ui
                    for _ in range(take):
                        if ui < len(units):
                            units[ui]()
                            ui += 1
                for f in fl[kvpc:]:
                    if f[0] in (opa, opb):
                        f[0](f[1])
                    else:
                        fn, m, off, sz = f
                        fn(m, off, sz)
                at_prev = ats[h]

            def oproj(qt):
                po_ = ps.tile([P, 512], F32, tag="proj")
                nc.tensor.matmul(po_[:], xsT2[3][:, qt * P:(qt + 1) * P],
                                 wo[3][:, :], start=True, stop=True)
                osb = opool.tile([P, D], F32, tag="osb")
                nc.vector.tensor_add(osb[:], po_[:], oacc[qt][:])
                nc.sync.dma_start(out_d[qt * P:(qt + 1) * P, :], osb[:])

            # last head: output projection of each query-half starts as soon
            # as that half's softmax normalize lands
            # last head: qh1 attnV matmuls follow qh0's immediately (no PE
            # bubble waiting on the qh0 normalize); both norm chains queue on
            # DVE ahead of the output-projection adds
            last = list(attnv_units(H - 1, at_prev))
            for emit in last[0:kvpc]:          # qh0 mms
                emit()
            last[kvpc]()                       # norm qh0
            for emit in last[kvpc + 1:2 * kvpc + 1]:  # qh1 mms
                emit()
            last[-1]()                         # norm qh1
            for qt in range(LQ // P):
                oproj(qt)

    nc.compile()
    return nc


def _host_inputs(query, key, value, mask, Wq, bq, Wk, bk, Wv, bv, Wo, bo):
    """Build the 8 per-core input maps; returns (in_maps, kvpc)."""
    f32, bf16 = np.float32, ml_dtypes.bfloat16
    idxs = [np.flatnonzero(mask[b]) for b in range(B)]
    cnts = [len(ix) for ix in idxs]
    kvpc = max(2, math.ceil(max(cnts) / P))
    kvp = kvpc * P

    def m_major(W):  # [p, m, kc*128+mc]: W[kc*128+p, m*128+mc]
        return np.ascontiguousarray(
            np.asarray(W).reshape(MC, P, MC, P).transpose(1, 2, 0, 3)
            .reshape(P, MC, MC * P)).astype(bf16)

    def p_major(W):  # [p, kc, j]: W[kc*128+p, j]
        return np.ascontiguousarray(
            np.asarray(W).reshape(MC, P, -1).transpose(1, 0, 2)).astype(bf16)

    wq_ = m_major(Wq)
    wk_ = m_major(Wk)
    wv_ = p_major(Wv)
    wo_ = p_major(Wo)
    bq_ = np.ascontiguousarray(bq.astype(f32).reshape(MC, P).T)
    bk_ = np.ascontiguousarray(bk.astype(f32).reshape(MC, P).T)
    bv_ = bv.astype(f32).reshape(1, D)
    bo_ = bo.astype(f32).reshape(1, D)

    per_batch = []
    for b in range(B):
        cnt = cnts[b]
        xk = np.zeros((kvp, D), f32)
        xv = np.zeros((kvp, D), f32)
        xk[:cnt] = key[b][idxs[b]]
        xv[:cnt] = value[b][idxs[b]]
        xkT = p_major(xk.T)
        xvT = p_major(xv.T)
        mbias = np.where(np.arange(kvp) < cnt, EXP_SHIFT, MASK_BIAS)
        mb_ = np.ascontiguousarray(mbias.astype(f32).reshape(kvpc, P).T)
        per_batch.append((xkT, xvT, mb_))

    in_maps = []
    for c in range(N_CORES):
        b, half = c // 2, c % 2
        sl = slice(half * LQ, (half + 1) * LQ)
        xqT = p_major(query[b, sl, :].T)
        xkT, xvT, mb_ = per_batch[b]
        in_maps.append({
            "xqT": xqT, "xkT": xkT, "xvT": xvT,
            "wq": wq_, "wk": wk_, "wv": wv_, "wo": wo_,
            "bq": bq_, "bk": bk_, "bv": bv_, "bo": bo_, "mb": mb_,
        })
    return in_maps, kvpc


def kernel(query, key, value, mask, Wq, bq, Wk, bk, Wv, bv, Wo, bo):
    in_maps, kvpc = _host_inputs(query, key, value, mask,
                                 Wq, bq, Wk, bk, Wv, bv, Wo, bo)
    if kvpc not in _cache:
        _cache[kvpc] = _build(kvpc)
    nc = _cache[kvpc]
    res = run_bass_kernel_spmd(nc, in_maps, list(range(N_CORES))).results
    out = np.empty((B, L, D), np.float32)
    for c in range(N_CORES):
        b, half = c // 2, c % 2
        out[b, half * LQ:(half + 1) * LQ, :] = res[c]["out"]
    return out


# revision 42
# speedup vs baseline: 1.0039x; 1.0039x over previous
"""Multi-head attention (B=4, L=2048, D=512, H=8) on 8 Trainium2 cores.

Sharding: core c handles batch b = c//2, query rows [(c%2)*1024, +1024).
The key-padding mask knocks out ~half of all kv positions, so the host
compresses K/V to the unmasked positions only (padded to a multiple of
128, kvpc chunks); each core projects the full compressed K/V for its
batch (no collectives).  All matmuls are bf16 (fp8 attn was tried: its
~2.4% rms quantization error lands right at the 2e-2 tolerance).

Key scheduling facts (measured on hw):
 - matmul time = N(out free) cycles only; K=64 matmuls carry a ~100ns
   penalty, so scores use per-head zero-padded q (qTz) for full K=128
 - the exp ACT ops (~1.07us per [128,1024] chunk) and the tensor queue
   are co-critical: projections are emitted as deadline-placed filler
   between score/attnV units so neither engine starves
 - each dma_start costs ~620ns of sync-queue issue; inputs are a few
   host-pre-laid [128, chunk, cols] tensors
 - output projection is split hp0+hp1 (filler, window 5-6) / hp2
   (window 7) / hp3 (tail) via an SBUF accumulator

Device layouts (per core):
  xq/xk/xv_all (128, 4, LQ|KVP)  inputs, dmodel chunk on dim 1
  qTz (128, LQ) x8               head h's q at partitions 64*(h%2), rest 0
  kT (128, KVP) x4               head pair hp at partition halves
  V (128, 8*128) per kv chunk    per head: 64 V cols then 64 ones cols
                                 (denominator rows come out replicated)
  at (128, 1024) bf16            exp(attn) for one kv chunk
  ss (128kv, 1024q) PSUM         scores; exp+mask+scale in one ACT op
  xs (128, 512) PSUM             attn@V accum; rows 64-127 = softmax denom
"""
import math

import numpy as np
import ml_dtypes

import concourse.bacc as bacc
import concourse.bass as bass
import concourse.mybir as mybir
import concourse.tile as tile
from concourse.bass_utils import run_bass_kernel_spmd

F32 = mybir.dt.float32
BF16 = mybir.dt.bfloat16
AF = mybir.ActivationFunctionType

B, L, D = 4, 2048, 512
H, DK = 8, 64
N_CORES = 8
LQ = L // 2            # query rows per core
P = 128
MC = D // P            # 4 dmodel chunks
MASK_BIAS = np.float32(-1e30)
EXP_SHIFT = np.float32(0.0)

_cache = {}


def _build(kvpc):
    """kvpc = number of 128-wide kv chunks after mask compression."""
    kvp = kvpc * P

    nc = bacc.Bacc("TRN2", target_bir_lowering=False, debug=False,
                   num_devices=N_CORES)

    # inputs are host-laid-out partition-major ([128, chunk, cols]) so each
    # loads with ONE dma_start (sync-queue issue slots are ~620ns each);
    # wq/wk are additionally m-major so the m=0 slice can load first
    xqT_d = nc.dram_tensor("xqT", [P, MC, LQ], BF16, kind="ExternalInput").ap()
    xkT_d = nc.dram_tensor("xkT", [P, MC, kvp], BF16, kind="ExternalInput").ap()
    xvT_d = nc.dram_tensor("xvT", [P, MC, kvp], BF16, kind="ExternalInput").ap()
    wq_d = nc.dram_tensor("wq", [P, MC, D], BF16, kind="ExternalInput").ap()
    wk_d = nc.dram_tensor("wk", [P, MC, D], BF16, kind="ExternalInput").ap()
    wv_d = nc.dram_tensor("wv", [P, MC, D], BF16, kind="ExternalInput").ap()
    wo_d = nc.dram_tensor("wo", [P, MC, D], BF16, kind="ExternalInput").ap()
    bq_d = nc.dram_tensor("bq", [P, MC], F32, kind="ExternalInput").ap()
    bk_d = nc.dram_tensor("bk", [P, MC], F32, kind="ExternalInput").ap()
    bv_d = nc.dram_tensor("bv", [1, D], F32, kind="ExternalInput").ap()
    bo_d = nc.dram_tensor("bo", [1, D], F32, kind="ExternalInput").ap()
    mb_d = nc.dram_tensor("mb", [P, kvpc], F32, kind="ExternalInput").ap()
    out_d = nc.dram_tensor("out", [LQ, D], F32, kind="ExternalOutput").ap()

    kblocks = []
    off = 0
    while off < kvp:
        sz = min(512, kvp - off)
        kblocks.append((off, sz))
        off += sz

    with tile.TileContext(nc) as tc:
        with tc.tile_pool(name="const", bufs=1) as cpool, \
             tc.tile_pool(name="xin", bufs=1) as xpool, \
             tc.tile_pool(name="proj", bufs=1) as prpool, \
             tc.tile_pool(name="attn", bufs=3) as apool, \
             tc.tile_pool(name="norm", bufs=4) as npool, \
             tc.tile_pool(name="outp", bufs=3) as opool, \
             tc.tile_pool(name="ps", bufs=2, space="PSUM") as ps:

            # tiles declared up front; DMAs emitted in first-use order below
            wq_all = cpool.tile([P, MC, D], BF16, name="wq_all")
            wk_all = cpool.tile([P, MC, D], BF16, name="wk_all")
            wv_all = cpool.tile([P, MC, D], BF16, name="wv_all")
            wo_all = cpool.tile([P, MC, D], BF16, name="wo_all")
            xq_all = xpool.tile([P, MC, LQ], BF16, name="xq_all")
            xk_all = xpool.tile([P, MC, kvp], BF16, name="xk_all")
            xv_all = xpool.tile([P, MC, kvp], BF16, name="xv_all")
            xqT = [xq_all[:, kc, :] for kc in range(MC)]
            xkT = [xk_all[:, kc, :] for kc in range(MC)]
            xvT = [xv_all[:, kc, :] for kc in range(MC)]
            wv = [wv_all[:, kc, :] for kc in range(MC)]
            wo = [wo_all[:, kc, :] for kc in range(MC)]
            wqm = [wq_all[:, m, :] for m in range(MC)]
            wkm = [wk_all[:, m, :] for m in range(MC)]

            nc.sync.dma_start(wq_all[:, 0, :], wq_d[:, 0, :])
            nc.sync.dma_start(xq_all[:, :, 0:512], xqT_d[:, :, 0:512])
            nc.sync.dma_start(xq_all[:, :, 512:1024], xqT_d[:, :, 512:1024])
            bq = cpool.tile_from(bq_d)
            bk = cpool.tile_from(bk_d)
            mb = cpool.tile_from(mb_d)
            nc.sync.dma_start(wk_all[:, 0, :], wk_d[:, 0, :])
            nc.sync.dma_start(xk_all[:, :, 0:512], xkT_d[:, :, 0:512])
            if kvp > 512:
                nc.sync.dma_start(xk_all[:, :, 512:kvp], xkT_d[:, :, 512:kvp])
            nc.sync.dma_start(wv_all[:], wv_d)
            nc.sync.dma_start(xv_all[:], xvT_d)
            nc.sync.dma_start(wq_all[:, 1:MC, :], wq_d[:, 1:MC, :])
            nc.sync.dma_start(wk_all[:, 1:MC, :], wk_d[:, 1:MC, :])
            bv = cpool.tile_from(bv_d)
            bo = cpool.tile_from(bo_d)
            bv_bc = cpool.tile([P, D], F32)
            nc.gpsimd.partition_broadcast(bv_bc[:], bv[:])
            bo_bc = cpool.tile([P, D], F32)
            nc.gpsimd.partition_broadcast(bo_bc[:], bo[:])
            nc.sync.dma_start(wo_all[:], wo_d)

            qTz = [prpool.tile([P, LQ], BF16, tag=f"qz{h}", name=f"qz{h}")
                   for h in range(H)]
            zeroed = set()
            kT = [prpool.tile([P, kvp], BF16, tag=f"kT{m}", name=f"kT{m}")
                  for m in range(MC)]
            V = [prpool.tile([P, H * P], BF16, tag=f"V{t}", name=f"V{t}")
                 for t in range(kvpc)]
            xsT2 = [prpool.tile([P, LQ], BF16, tag=f"xs{hp}",
                                name=f"xsT2_{hp}") for hp in range(MC)]

            def qproj_blk(m, off, sz):
                if m not in zeroed:  # zero the other head's half once
                    zeroed.add(m)
                    nc.vector.memset(qTz[2 * m][64:128, :], 0.0)
                    nc.vector.memset(qTz[2 * m + 1][0:64, :], 0.0)
                wv4 = wqm[m].rearrange("p (kc mc) -> p kc mc", mc=P)
                pp = ps.tile([P, 512], F32, tag="proj")
                for kc in range(MC):
                    nc.tensor.matmul(
                        pp[:, 0:sz], wv4[:, kc, :],
                        xqT[kc][:, off:off + sz],
                        start=kc == 0, stop=kc == MC - 1)
                nc.vector.tensor_scalar_add(qTz[2 * m][0:64, off:off + sz],
                                            pp[0:64, 0:sz], bq[0:64, m:m + 1])
                nc.vector.tensor_scalar_add(
                    qTz[2 * m + 1][64:128, off:off + sz],
                    pp[64:128, 0:sz], bq[64:128, m:m + 1])

            def kproj_blk(m, off, sz):
                wv4 = wkm[m].rearrange("p (kc mc) -> p kc mc", mc=P)
                pp = ps.tile([P, 512], F32, tag="proj")
                for kc in range(MC):
                    nc.tensor.matmul(
                        pp[:, 0:sz], wv4[:, kc, :],
                        xkT[kc][:, off:off + sz],
                        start=kc == 0, stop=kc == MC - 1)
                nc.vector.tensor_scalar_add(kT[m][:, off:off + sz],
                                            pp[:, 0:sz], bk[:, m:m + 1])

            def qproj(m):
                for off, sz in ((0, 512), (512, 512)):
                    qproj_blk(m, off, sz)

            def kproj(m):
                for off, sz in kblocks:
                    kproj_blk(m, off, sz)

            def vproj(t):
                pv = ps.tile([P, 512], F32, tag="proj")
                for kc in range(MC):
                    nc.tensor.matmul(pv[:], xvT[kc][:, t * P:(t + 1) * P],
                                     wv[kc][:, :], start=kc == 0,
                                     stop=kc == MC - 1)
                v8 = V[t].rearrange("p (g d) -> p g d", d=P)
                nc.vector.tensor_add(v8[:, :, 0:64],
                                     pv.rearrange("p (g d) -> p g d", d=64),
                                     bv_bc.rearrange("p (g d) -> p g d", d=64))
                nc.vector.memset(v8[:, :, 64:P], 1.0)

            def score_chunk(h, c, at):
                hp, po = h // 2, 64 * (h % 2)
                ss = ps.tile([P, 1024], F32, tag="scores")
                for qh in range(2):
                    nc.tensor.matmul(
                        ss[:, qh * 512:(qh + 1) * 512],
                        kT[hp][:, c * P:(c + 1) * P],
                        qTz[h][:, qh * 512:(qh + 1) * 512],
                        start=True, stop=True)
                nc.scalar.activation(at[c][:], ss[:], AF.Exp,
                                     bias=mb[:, c:c + 1], scale=0.125)

            def attnv_units(h, at):
                """Yield emission thunks for attn@V + normalize of head h."""
                hp, po = h // 2, 64 * (h % 2)
                xs = [None, None]

                def mk_mm(qh, c):
                    def emit():
                        if c == 0:
                            xs[qh] = ps.tile([P, 512], F32, tag="xs",
                                             name=f"xs_h{h}_{qh}")
                        nc.tensor.matmul(
                            xs[qh][:], V[c][:, P * h:P * h + P],
                            at[c][:, qh * 512:(qh + 1) * 512],
                            start=c == 0, stop=c == kvpc - 1)
                    return emit

                def mk_norm(qh):
                    def emit():
                        srow = npool.tile([64, 512], F32, tag="srow")
                        nc.vector.tensor_copy(srow[:], xs[qh][64:128, :])
                        rec = npool.tile([64, 512], F32, tag="rec")
                        nc.vector.reciprocal_approx_fast(rec[:], srow[:])
                        nc.vector.tensor_mul(
                            xsT2[hp][po:po + 64, qh * 512:(qh + 1) * 512],
                            xs[qh][0:64, :], rec[:])
                    return emit

                for qh in range(2):
                    for c in range(kvpc):
                        yield mk_mm(qh, c)
                    yield mk_norm(qh)

            def new_at(h):
                return [apool.tile([P, 1024], BF16, tag=f"at{c}",
                                   name=f"at_h{h}_{c}")
                        for c in range(kvpc)]

            # ---- emission schedule ----
            # head 0: kproj blocks land just before the score chunks that
            # need them.  The first chunk's scores+exp are split by q-half
            # so the exp engine starts before the second q-half projects.
            ats = {0: new_at(0)}
            qproj_blk(0, 0, 512)
            kproj_blk(0, *kblocks[0])
            ss0 = ps.tile([P, 1024], F32, tag="scores")
            nc.tensor.matmul(ss0[:, 0:512], kT[0][:, 0:P],
                             qTz[0][:, 0:512], start=True, stop=True)
            nc.scalar.activation(ats[0][0][:, 0:512], ss0[:, 0:512], AF.Exp,
                                 bias=mb[:, 0:1], scale=0.125)
            qproj_blk(0, 512, 512)
            nc.tensor.matmul(ss0[:, 512:1024], kT[0][:, 0:P],
                             qTz[0][:, 512:1024], start=True, stop=True)
            nc.scalar.activation(ats[0][0][:, 512:1024], ss0[:, 512:1024],
                                 AF.Exp, bias=mb[:, 0:1], scale=0.125)
            nb = kblocks[0][0] + kblocks[0][1]
            for c in range(1, kvpc):
                if c * P >= nb:  # next kproj block
                    bi = len([1 for o, s in kblocks if o < nb])
                    off, sz = kblocks[bi]
                    kproj_blk(0, off, sz)
                    nb = off + sz
                score_chunk(0, c, ats[0])

            # output projection is split: hp0/hp1 partials run as mid-kernel
            # filler into an SBUF accumulator (bias folded in); only hp2/hp3
            # + one DVE add remain for the tail
            oacc = [prpool.tile([P, D], F32, tag=f"oacc{qt}",
                                name=f"oacc{qt}") for qt in range(LQ // P)]

            def opa(qt):
                pA = ps.tile([P, 512], F32, tag="proj")
                for hp in (0, 1):
                    nc.tensor.matmul(pA[:], xsT2[hp][:, qt * P:(qt + 1) * P],
                                     wo[hp][:, :], start=hp == 0, stop=hp == 1)
                nc.vector.tensor_add(oacc[qt][:], pA[:], bo_bc[:])

            def opb(qt):
                pB = ps.tile([P, 512], F32, tag="proj")
                nc.tensor.matmul(pB[:], xsT2[2][:, qt * P:(qt + 1) * P],
                                 wo[2][:, :], start=True, stop=True)
                nc.vector.tensor_add(oacc[qt][:], pB[:], oacc[qt][:])

            def mk_qk(m):
                return ([(qproj_blk, m, 0, 512), (qproj_blk, m, 512, 512)] +
                        [(kproj_blk, m, o, s) for o, s in kblocks])

            # filler placed by deadline: qk(m) lands a window before the
            # heads that read it; oproj partials go late (after head 3)
            ats[1] = new_at(1)
            qk1 = mk_qk(1)
            for c in range(kvpc):
                score_chunk(1, c, ats[1])
                vproj(c)
                if c < len(qk1):
                    fn, m, off, sz = qk1[c]
                    fn(m, off, sz)
            for f in qk1[kvpc:]:
                fn, m, off, sz = f
                fn(m, off, sz)

            filler = {2: mk_qk(2), 4: mk_qk(3),
                      5: [(opa, qt) for qt in range(4)],
                      6: [(opa, qt) for qt in range(4, 8)],
                      7: [(opb, qt) for qt in range(8)]}

            for h in range(1, H):
                if h < H - 1:
                    ats[h + 1] = new_at(h + 1)
                units = list(attnv_units(h - 1, ats[h - 1]))
                fl = filler.get(h, [])
                ui = 0
                for c in range(kvpc):
                    if c < len(fl):
                        if fl[c][0] in (opa, opb):
                            fl[c][0](fl[c][1])
                        else:
                            fn, m, off, sz = fl[c]
                            fn(m, off, sz)
                    if h < H - 1:
                        score_chunk(h + 1, c, ats[h + 1])
                    take = 2 if c < kvpc - 1 else len(units) - ui
                    for _ in range(take):
                        if ui < len(units):
                            units[ui]()
                            ui += 1
                for f in fl[kvpc:]:
                    if f[0] in (opa, opb):
                        f[0](f[1])
                    else:
                        fn, m, off, sz = f
                        fn(m, off, sz)
                at_prev = ats[h]

            def oproj(qt):
                po_ = ps.tile([P, 512], F32, tag="proj")
                nc.tensor.matmul(po_[:], xsT2[3][:, qt * P:(qt + 1) * P],
                                 wo[3][:, :], start=True, stop=True)
                osb = opool.tile([P, D], F32, tag="osb")
                nc.vector.tensor_add(osb[:], po_[:], oacc[qt][:])
                nc.sync.dma_start(out_d[qt * P:(qt + 1) * P, :], osb[:])

            # last head: output projection of each query-half starts as soon
            # as that half's softmax normalize lands
            # last head: qh1 attnV matmuls follow qh0's immediately (no PE
            # bubble waiting on the qh0 normalize); both norm chains queue on
            # DVE ahead of the output-projection adds
            last = list(attnv_units(H - 1, at_prev))
            for emit in last[0:kvpc]:          # qh0 mms
                emit()
            last[kvpc]()                       # norm qh0
            for emit in last[kvpc + 1:2 * kvpc + 1]:  # qh1 mms
                emit()
            last[-1]()                         # norm qh1
            for qt in range(LQ // P):
                oproj(qt)

    nc.compile()
    return nc


def _host_inputs(query, key, value, mask, Wq, bq, Wk, bk, Wv, bv, Wo, bo):
    """Build the 8 per-core input maps; returns (in_maps, kvpc)."""
    f32, bf16 = np.float32, ml_dtypes.bfloat16
    idxs = [np.flatnonzero(mask[b]) for b in range(B)]
    cnts = [len(ix) for ix in idxs]
    kvpc = max(2, math.ceil(max(cnts) / P))
    kvp = kvpc * P

    def m_major(W):  # [p, m, kc*128+mc]: W[kc*128+p, m*128+mc]
        return np.ascontiguousarray(
            np.asarray(W).reshape(MC, P, MC, P).transpose(1, 2, 0, 3)
            .reshape(P, MC, MC * P)).astype(bf16)

    def p_major(W):  # [p, kc, j]: W[kc*128+p, j]
        return np.ascontiguousarray(
            np.asarray(W).reshape(MC, P, -1).transpose(1, 0, 2)).astype(bf16)

    wq_ = m_major(Wq)
    wk_ = m_major(Wk)
    wv_ = p_major(Wv)
    wo_ = p_major(Wo)
    bq_ = np.ascontiguousarray(bq.astype(f32).reshape(MC, P).T)
    bk_ = np.ascontiguousarray(bk.astype(f32).reshape(MC, P).T)
    bv_ = bv.astype(f32).reshape(1, D)
    bo_ = bo.astype(f32).reshape(1, D)

    per_batch = []
    for b in range(B):
        cnt = cnts[b]
        xk = np.zeros((kvp, D), f32)
        xv = np.zeros((kvp, D), f32)
        xk[:cnt] = key[b][idxs[b]]
        xv[:cnt] = value[b][idxs[b]]
        xkT = p_major(xk.T)
        xvT = p_major(xv.T)
        mbias = np.where(np.arange(kvp) < cnt, EXP_SHIFT, MASK_BIAS)
        mb_ = np.ascontiguousarray(mbias.astype(f32).reshape(kvpc, P).T)
        per_batch.append((xkT, xvT, mb_))

    in_maps = []
    for c in range(N_CORES):
        b, half = c // 2, c % 2
        sl = slice(half * LQ, (half + 1) * LQ)
        xqT = p_major(query[b, sl, :].T)
        xkT, xvT, mb_ = per_batch[b]
        in_maps.append({
            "xqT": xqT, "xkT": xkT, "xvT": xvT,
            "wq": wq_, "wk": wk_, "wv": wv_, "wo": wo_,
            "bq": bq_, "bk": bk_, "bv": bv_, "bo": bo_, "mb": mb_,
        })
    return in_maps, kvpc


def kernel(query, key, value, mask, Wq, bq, Wk, bk, Wv, bv, Wo, bo):
    in_maps, kvpc = _host_inputs(query, key, value, mask,
                                 Wq, bq, Wk, bk, Wv, bv, Wo, bo)
    if kvpc not in _cache:
        _cache[kvpc] = _build(kvpc)
    nc = _cache[kvpc]
    res = run_bass_kernel_spmd(nc, in_maps, list(range(N_CORES))).results
    out = np.empty((B, L, D), np.float32)
    for c in range(N_CORES):
        b, half = c // 2, c % 2
        out[b, half * LQ:(half + 1) * LQ, :] = res[c]["out"]
    return out
